# revision 52
# baseline (speedup 1.0000x reference)
"""CT forward projector (3D, axis-aligned +z rays) on 8 TRN2 NeuronCores.

Geometry (fixed by the problem): M = I, b = 0, rays travel along +z with
constant (x, y). Hence each ray reads a single contiguous z-column
volume[i, j, :] (i = round(x), j = round(y)), and for segment m with
midpoint mid_m = 0.5*(a_m + a_{m+1}) (a = 257*t - 1) the reference adds
volume[i, j, round(mid_m)] * (a_{m+1} - a_m), dropping out-of-range bins.

Device algorithm per ray (per SBUF partition), exploiting sorted t:
  rr_m = RNE(ah_m + ah_{m+1}) + 1 in [0, 257]     (ah = 128.5*t - 0.5)
  rr is non-decreasing, so equal bins form contiguous runs.
  scatter1 (gpsimd local_scatter, duplicate indices resolve last-wins on
    HW): S1[rr_m] = m + 1  ->  S1[z] = (last segment of run z) + 1.
  scatter2: C1[S1[z]] = row[z] where row is the ray's padded f16 slab row
    [1e-4, col(0..255), 1e-4] (pads absorb the out-of-range bins; empty
    bins collide harmlessly on the never-read slot 0).
  backward hold-last fill (tensor_tensor_scan over reversed APs):
    state = (C1==0)*state + C1  ->  cfill[m+1] = col[bin(m)] (0 if oob).
  out_ray = sum_m cfill[m+1] * (ah_{m+1} - ah_m) * 2.

Work is batched 4 ray-tiles (512 rays) per engine op; ops are placed so
DVE / ACT(scalar) / Pool(gpsimd) run near-balanced, with the column
dma_gathers (mlp gpsimd library) grouped before all local_scatters
(local_scatter library) so Bacc inserts exactly two ucode reloads.

Sharding: rays sorted by (i, j) = round(x), round(y), split into 8 equal
shards of 8192; each core receives its rays' t rows plus the x-slab of
the volume covering its (i, j) range (f16, rows padded to 384), and
dma_gathers its 8192 z-columns from DRAM.
"""

import sys

sys.path.insert(0, "/opt/trn_rl_repo")

import numpy as np

N_RAY = 65536
K = 256
NXYZ = 256
N_CORES = 8
RPC = N_RAY // N_CORES          # rays per core = 8192
TILES = RPC // 128              # 64 ray-tiles per core
# chunk sizes in quads (1 quad = 512 rays): small leading chunks so the
# first quad's columns land before the bulk of the gather.
CHUNK_QUADS = [1, 1, 2, 2, 2, 2, 2, 2, 2]
CHUNKS = len(CHUNK_QUADS)
SLAB_PLANES = 48                # x-planes shipped per core (span is ~33)
SLAB_ROWS = SLAB_PLANES * NXYZ  # 12288 (< 32767 so rows index as int16)
SLAB_W = 384                    # padded f16 row: [pad, col(256), pad, 0...]
PADV = 1e-4                     # sentinel for out-of-range bins
NSEG = K - 1                    # 255 segments
NBINS = K + 2                   # 258 scatter-1 bins: rr in [0, 257]
MAGIC = 12582912.0              # 1.5 * 2**23: RNE-to-integer via add/sub

_BUILT = {}


def _build_bass(nbuf=4):
    import concourse.bass as bass
    import concourse.bacc as bacc
    import concourse.mybir as mybir
    from concourse.tile import TileContext

    f32 = mybir.dt.float32
    f16 = mybir.dt.float16
    i16 = mybir.dt.int16
    Alu = mybir.AluOpType
    Act = mybir.ActivationFunctionType

    NB_BRIDGE = 8             # dah/idx1 slots bridging pre -> post stages
    NQUADS = sum(CHUNK_QUADS)
    QT = 4                    # sub-tiles (128-ray groups) per op batch
    QF = QT * K               # 1024: batched free width
    QB = QT * NBINS           # 1032

    def _rev(ap):
        return bass.AP(
            ap.tensor,
            ap.offset + (ap.ap[-1][1] - 1) * ap.ap[-1][0],
            [list(ap.ap[0]), [-ap.ap[-1][0], ap.ap[-1][1]]],
        )

    nc = bacc.Bacc("TRN2", target_bir_lowering=False, debug=False)

    t_d = nc.dram_tensor("t", [RPC, K], f32, kind="ExternalInput")
    slab_d = nc.dram_tensor("slab", [SLAB_ROWS, SLAB_W], f16, kind="ExternalInput")
    gidx_d = nc.dram_tensor("gidx", [128, RPC // 16], i16, kind="ExternalInput")
    iota_d = nc.dram_tensor("iota", [128, K], i16, kind="ExternalInput")
    out_d = nc.dram_tensor("out", [128, TILES], f32, kind="ExternalOutput")

    with TileContext(nc) as tc:
        with (
            tc.tile_pool(name="const", bufs=1) as cpool,
            tc.tile_pool(name="tch", bufs=3) as tch_pool,
            tc.tile_pool(name="colch", bufs=1) as colch_pool,
            tc.tile_pool(name="pre", bufs=3) as prepool,
            tc.tile_pool(name="bridge", bufs=NB_BRIDGE) as brpool,
            tc.tile_pool(name="post", bufs=3) as wpool,
            tc.tile_pool(name="junkp", bufs=2) as jpool,
            tc.tile_pool(name="preset", bufs=1) as ppool,
        ):
            gidx = cpool.tile([128, RPC // 16], i16, tag="gidx")
            iota = cpool.tile([128, K], i16, tag="iota")
            out_sb = cpool.tile([128, TILES], f32, tag="out_sb")
            nc.sync.dma_start(out=gidx[:, :], in_=gidx_d[:, :])
            nc.sync.dma_start(out=iota[:, :], in_=iota_d[:, :])

            # Manually-rotated idx1 tiles: pad cols (m=255 of each
            # sub-tile) preset to -1, never rewritten.
            idx1_tiles = []
            for r in range(NB_BRIDGE):
                ix = ppool.tile([128, QF], i16, tag=f"idx1_{r}")
                ixa = ix[:, :]
                nc.vector.memset(
                    bass.AP(ixa.tensor, ixa.offset + K - 1,
                            [list(ixa.ap[0]), [K, QT], [1, 1]]),
                    -1,
                )
                idx1_tiles.append(ix)

            # All column gathers up-front: every InstDMAGatherAnt (mlp gpsimd
            # library) precedes every InstLocalScatter (local_scatter
            # library), so Bacc inserts exactly two library reloads.
            col_tiles = []
            ray0 = 0
            for ch, cq in enumerate(CHUNK_QUADS):
                nrays = cq * QT * 128
                col_ch = colch_pool.tile([128, cq * QT, SLAB_W], f16, tag=f"col{ch}")
                nc.gpsimd.dma_gather(
                    out_ap=col_ch[:, :, :],
                    in_ap=slab_d.ap(),
                    idxs_ap=gidx[:, ray0 // 16 : (ray0 + nrays) // 16],
                    num_idxs=nrays,
                    num_idxs_reg=nrays,
                    elem_size=SLAB_W,
                )
                col_tiles.append(col_ch)
                ray0 += nrays

            qi = 0
            for ch, cq in enumerate(CHUNK_QUADS):
                col_ch = col_tiles[ch]
                for q in range(cq):
                    idx1 = idx1_tiles[qi % NB_BRIDGE]
                    t_qt = tch_pool.tile([128, QT, K], f32, tag="t_q")
                    # t rows for this quad: ray g*128+p on partition p, group g.
                    # Quad 0 loads per sub-tile so its first compute starts as
                    # soon as the first 128 rays land.
                    nsub_ld = QT if qi == 0 else 1
                    for sl in range(nsub_ld):
                        w = QT // nsub_ld
                        nc.sync.dma_start(
                            out=t_qt[:, sl * w : (sl + 1) * w, :],
                            in_=bass.AP(
                                t_d,
                                (qi * QT + sl * w) * 128 * K,
                                [[K, 128], [128 * K, w], [1, K]],
                            ),
                        )
                    t_q = t_qt[:, :, :]

                    ah = prepool.tile([128, QF], f32, tag="ah")
                    y = prepool.tile([128, QF - 1], f32, tag="y")
                    sp = prepool.tile([128, QF - 1], f32, tag="sp")
                    dah = brpool.tile([128, QF - 1], f32, tag="dah")
                    s1 = wpool.tile([128, QT, NBINS], i16, tag="s1")
                    c1 = wpool.tile([128, QF], f16, tag="c1")
                    gg = wpool.tile([128, QF], f16, tag="gg")
                    cfill = wpool.tile([128, QF], f16, tag="cfill")
                    junk = jpool.tile([128, NSEG], f32, tag="junk")

                    # ah = 128.5 * t - 0.5 (ACT); S' = ah_j + ah_{j+1} (DVE,
                    # sub-tile boundary positions are garbage, skipped later).
                    # Quad 0 runs both per sub-tile to shorten the pipeline
                    # fill; later quads in one call.
                    for sl in range(nsub_ld):
                        w = QT // nsub_ld
                        ah3 = bass.AP(
                            ah[:, :].tensor, ah[:, :].offset + sl * w * K,
                            [list(ah[:, :].ap[0]), [K, w], [1, K]],
                        )
                        nc.scalar.activation(
                            out=ah3, in_=t_q[:, sl * w : (sl + 1) * w, :],
                            func=Act.Copy, bias=-0.5, scale=128.5,
                        )
                        lo = sl * w * K
                        # stop before the sub-group pad so no read crosses
                        # into the next sub-tile's ah (pads are never read)
                        hi = min((sl + 1) * w * K - 1, QF - 1)
                        nc.vector.tensor_tensor(
                            out=sp[:, lo:hi], in0=ah[:, lo:hi],
                            in1=ah[:, lo + 1 : hi + 1], op=Alu.add,
                        )
                    # idx1 = RNE(S') + 1 as int16 via two ACT Copy stages
                    # (y = S' + MAGIC rounds to integer; then - (MAGIC-1)),
                    # written through 3D views that skip the pad columns
                    # (preset -1 in the prologue).
                    nc.scalar.activation(
                        out=y[:, :], in_=sp[:, :], func=Act.Copy,
                        bias=MAGIC, scale=1.0,
                    )
                    ixa = idx1[:, :]
                    ya = y[:, :]
                    nc.scalar.activation(
                        out=bass.AP(ixa.tensor, ixa.offset,
                                    [list(ixa.ap[0]), [K, QT], [1, NSEG]]),
                        in_=bass.AP(ya.tensor, ya.offset,
                                    [list(ya.ap[0]), [K, QT], [1, NSEG]]),
                        func=Act.Copy, bias=-(MAGIC - 1.0), scale=1.0,
                    )
                    # dah = ah_{j+1} - ah_j  (= seg_len / 2)
                    nc.vector.tensor_tensor(
                        out=dah[:, :], in0=ah[:, 1:QF], in1=ah[:, 0 : QF - 1],
                        op=Alu.subtract,
                    )
                    # scatter1 per sub-tile: S1[z] = (last segment of run z) + 1
                    for s in range(QT):
                        nc.gpsimd.local_scatter(
                            out_ap=s1[:, s, :], data_ap=iota[:, :],
                            idxs_ap=idx1[:, s * K : (s + 1) * K],
                            channels=128, num_elems=NBINS, num_idxs=K,
                        )
                    # scatter2 per sub-tile, 1-BASED positions: use S1 raw as
                    # indices, so C1[(last-of-run)+1] = col value. Empty bins
                    # (S1 = 0) collide on slot 0, which the dots never read;
                    # last-wins makes that harmless. The fill and the dot
                    # shift by one position accordingly.
                    for s in range(QT):
                        nc.gpsimd.local_scatter(
                            out_ap=c1[:, s * K : (s + 1) * K],
                            data_ap=col_ch[:, q * QT + s, 0:NBINS],
                            idxs_ap=s1[:, s, :],
                            channels=128, num_elems=K, num_idxs=NBINS,
                        )
                    # G = (C1 == 0): 1 unscattered / 0 scattered. Scattered
                    # cols are raw volume samples (clamped away from zero
                    # host-side); pad bins carry 1e-4 sentinels. Runs on Pool
                    # early (DVE is the global binder) but on DVE for the
                    # last quads, where Pool's scatter stream paces the tail.
                    # quad-wide flag (Pool) + one backward hold-last scan;
                    # every sub-tile's m=254 slot is scattered so state
                    # resets at each sub boundary (pads are never read).
                    nc.gpsimd.tensor_scalar(
                        out=gg[:, :], in0=c1[:, :], scalar1=0.0, scalar2=None,
                        op0=Alu.is_equal,
                    )
                    nc.vector.tensor_tensor_scan(
                        out=_rev(cfill[:, :]), data0=_rev(gg[:, :]),
                        data1=_rev(c1[:, :]), initial=0.0,
                        op0=Alu.mult, op1=Alu.add,
                    )
                    # out_ray = sum_m (c * 2) * dah   per sub-tile
                    for s in range(QT):
                        g = qi * QT + s  # global 128-ray tile index
                        nc.vector.scalar_tensor_tensor(
                            out=junk[:, 0:NSEG],
                            in0=cfill[:, s * K + 1 : s * K + 1 + NSEG], scalar=2.0,
                            in1=dah[:, s * K : s * K + NSEG],
                            op0=Alu.mult, op1=Alu.mult,
                            accum_out=out_sb[:, g : g + 1],
                        )
                    qi += 1

            nc.sync.dma_start(out=out_d[:, :], in_=out_sb[:, :])

    return nc


def _get_nc():
    if "nc" not in _BUILT:
        nc = _build_bass()
        nc.compile()
        _BUILT["nc"] = nc
    return _BUILT["nc"]


def _host_prep(volume, src, t_sorted):
    """Sort rays by (i, j); build per-core inputs."""
    vol = np.ascontiguousarray(np.asarray(volume, dtype=np.float32))
    src = np.asarray(src, dtype=np.float32)
    t = np.ascontiguousarray(np.asarray(t_sorted, dtype=np.float32))

    i = np.round(src[:, 0]).astype(np.int32)
    j = np.round(src[:, 1]).astype(np.int32)
    rowidx = i * NXYZ + j
    order = np.argsort(rowidx, kind="stable")

    vol_rows = vol.reshape(NXYZ * NXYZ, NXYZ)
    in_maps = []
    sels = []
    iota = np.zeros((128, K), dtype=np.int16)
    iota[:, 0:NSEG] = (np.arange(NSEG, dtype=np.int16) + 1)[None, :]
    for c in range(N_CORES):
        sel = order[c * RPC : (c + 1) * RPC]
        sels.append(sel)
        rows = rowidx[sel]
        i_lo = int(rows[0]) >> 8
        local = rows - i_lo * NXYZ
        assert local.min() >= 0 and local.max() < SLAB_ROWS, (
            f"slab span exceeded: {local.min()}..{local.max()}"
        )
        slab = np.zeros((SLAB_ROWS, SLAB_W), dtype=np.float16)
        hi = min(NXYZ * NXYZ, i_lo * NXYZ + SLAB_ROWS)
        n = hi - i_lo * NXYZ
        # clamp away from 0 so the scatter-occupancy flag (C1 == 0) never
        # misfires on an exactly-zero voxel (perturbs values by < 6.2e-5)
        slab[:n, 1 : NXYZ + 1] = np.maximum(
            vol_rows[i_lo * NXYZ : hi].astype(np.float16), np.float16(6.2e-5)
        )
        slab[:, 0] = PADV
        slab[:, NXYZ + 1] = PADV
        # dma_gather wrapped-16 index layout: index q at [q % 16, q // 16],
        # replicated across the 8 gpsimd 16-partition groups.
        gidx = np.zeros((128, RPC // 16), dtype=np.int16)
        gidx[0:16, :] = local.astype(np.int16).reshape(RPC // 16, 16).T
        for a in range(1, 8):
            gidx[16 * a : 16 * (a + 1), :] = gidx[0:16, :]
        in_maps.append(
            {
                "t": np.ascontiguousarray(t[sel]),
                "slab": slab,
                "gidx": gidx,
                "iota": iota,
            }
        )
    return in_maps, sels


def kernel(volume, M, b, src, dst, t_sorted):
    from concourse.bass_utils import run_bass_kernel_spmd

    in_maps, sels = _host_prep(volume, src, t_sorted)
    nc = _get_nc()
    res = run_bass_kernel_spmd(nc, in_maps, list(range(N_CORES)))
    outs = res.results
    full = np.zeros(N_RAY, dtype=np.float32)
    for c in range(N_CORES):
        o = np.asarray(outs[c]["out"])  # [128, TILES]; ray g*128+p at [p, g]
        full[sels[c]] = o.T.reshape(RPC)
    return full
